# revision 51
# baseline (speedup 1.0000x reference)
"""Trainium2 Bass kernel for AggregateSelfAttention (ragged clusters).

Math (reference):
    flat = mention_vectors.reshape(8192, 768)
    v[c,l,:] = flat[idx[c,l]]
    s[c,l]   = relu(v @ W1 + b1) @ Wout + bout
    p        = softmax(mask(s))
    out[c]   = sum_l p[c,l] * v[c,l]

Key restructurings (validated vs reference at ~3e-4 rel l2 in fp16):
  * The score s[c,l] depends only on the mention row -> compute the FFN once
    per table row (8192 rows, sharded 1024/core) instead of per (c,l)
    occurrence (32768 rows): 4x less matmul work.  bout drops out entirely
    (softmax shift invariance).
  * Unnormalized softmax: p = exp(s)*valid / sum_l exp(s)*valid.  exp is safe
    without max subtraction (|s| < ~5 for unit-normal data; fp32 exp).
  * Host builds an augmented fp16 table  aug[m] = [flat[m] | exp(s_m) | 1 | pad]
    (row = 896 fp16 = 1792B, 256B-aligned for dma_gather).  Padded (c,l) slots
    point at a sentinel all-zeros row, so no masking is needed on device: the
    zero "exp" and zero "1" columns contribute nothing to numerator or
    denominator.
  * Phase 2 gathers the compacted valid (c, l) slots (~2300 rows/core after
    host load-balancing of concepts across cores, vs 4096 naive) with SWDGE
    dma_gather (1792B/descriptor) and does the ragged weighted sums as
    masked PE matmuls: lhsT[128,128] = host-built 0/1 slot->concept mask x
    per-row exp(s); rhs = gathered rows; the "1" column yields the softmax
    denominator in the same matmul.  One DVE reciprocal+scale normalizes.

Sharding: concepts (2048 -> 256/core) for phase 2; table rows (8192 ->
1024/core) for phase 1.  No collectives: phase 1 outputs exp-scores which the
host concatenates into the phase-2 table (pure data movement between the two
NEFF dispatches).

Perf notes (from NTFF traces):
  * Host pre-tiles all phase-1 operands into [128, free] partition-major
    contiguous blocks so each DMA descriptor is multi-KB (the naive
    rearranged DMA ran at 86 GB/s on 1.5KB descriptors).
  * Junk warmup matmuls at kernel start (and gated on mid gather chunks in
    phase 2) keep the PE HAM clock at 2.4 GHz; without them every matmul runs
    at the cold 1.2 GHz rate.
  * The gather's SWDGE descriptor generation (~8.6 ns/row, serial on the Q7,
    plus a fixed ~7us engine preamble and ~8-10us one-time ucode IRAM load)
    is phase 2's floor; everything else hides under it.  Gather chunks are
    sized big->small so the final chunk's generation + transfer tail is
    short.
"""

import os
import sys

import numpy as np

for _p in ("/opt/trn_rl_repo", "/root/.axon_site/_ro/trn_rl_repo"):
    if os.path.isdir(_p) and _p not in sys.path:
        sys.path.insert(0, _p)

from concourse import bacc, bass, mybir, tile  # noqa: E402
from concourse.bass_utils import run_bass_kernel_spmd  # noqa: E402

dt = mybir.dt

N_CORES = 8
B, M, D, C, L = 1, 8192, 768, 2048, 16
MS = M // N_CORES            # 1024 table rows per core (phase 1)
CS = C // N_CORES            # 256 concepts per core (phase 2)
NI = CS * L                  # 4096 gathered rows per core
AUGW = 896                   # fp16 aug row: 768 X | exp | 1 | pad -> 1792B
SENT = M                     # sentinel row index (all zeros)
ET = D // 128                # 6 partition tiles of the 768 dim
NG = CS // 128               # 2 concept groups of 128

_PROGRAMS = {}


def _new_bass() -> bacc.Bacc:
    return bacc.Bacc(
        "TRN2",
        target_bir_lowering=False,
        debug=False,
        num_devices=N_CORES,
    )


def _warmup(nc, pool, psum_pool, n_mm):
    """Junk matmuls to push the PE HAM clock to 2.4 GHz while DMAs run.

    Two alternating PSUM banks so the junk matmuls pipeline instead of
    serializing on a same-bank WAW drain.
    """
    wt = pool.tile([128, 512], dt.float16, tag="warm")
    nc.vector.memset(wt[:], 0.0)
    jps = [
        psum_pool.tile([128, 512], dt.float32, name="jp0", tag="jp0"),
        psum_pool.tile([128, 512], dt.float32, name="jp1", tag="jp1"),
    ]
    for i in range(n_mm):
        nc.tensor.matmul(jps[i % 2][:], wt[:, 0:128], wt[:], start=True,
                         stop=True, skip_group_check=True)
    return wt, jps


def _build_phase1() -> bass.Bass:
    """Per-core: exp(relu(X_shard @ W1 + b1) @ Wout) for 1024 table rows.

    Host-pretiled inputs (partition-major, contiguous per partition):
      xt[c, p, t*512 + m] = X_shard[512c + m, 128t + p]      (c: m-half)
      w1[h, p, ((j*ET + dti)*128) + e] = W1[128*dti + p, 128*(3h+j) + e]
      b1[p, et] = b1[128*et + p];  wout[p, et] = Wout[128*et + p]
    """
    nc = _new_bass()
    xt = nc.declare_dram_parameter("xt", [2, 128, ET * 512], dt.float16, isOutput=False)
    w1 = nc.declare_dram_parameter("w1", [2, 128, 3 * ET * 128], dt.float16, isOutput=False)
    b1 = nc.declare_dram_parameter("b1", [128, ET], dt.float32, isOutput=False)
    wout = nc.declare_dram_parameter("wout", [128, ET], dt.float16, isOutput=False)
    exps = nc.declare_dram_parameter("exps", [1, MS], dt.float32, isOutput=True)

    with tile.TileContext(nc) as tc:
        with (
            tc.tile_pool(name="sb", bufs=1) as pool,
            tc.tile_pool(name="psh", bufs=3, space=bass.MemorySpace.PSUM) as psh,
            tc.tile_pool(name="pss", bufs=1, space=bass.MemorySpace.PSUM) as pss,
            tc.tile_pool(name="psj", bufs=1, space=bass.MemorySpace.PSUM) as psj,
        ):
            _warmup(nc, pool, psj, 12)

            w1_sb = pool.tile([128, ET, ET, 128], dt.float16)  # [p, et, dti, e]
            xT_sb = pool.tile([128, 2, ET, 512], dt.float16)   # [p, c, t, m]
            b1_sb = pool.tile([128, ET], dt.float32)
            wout_sb = pool.tile([128, ET], dt.float16)
            # order matters: first compute needs w1 half 0 + xt half 0.
            # Two HWDGE rings (sync + scalar) feed the SDMA engines in
            # parallel to cut the latency to the first matmul's operands.
            nc.sync.dma_start(out=w1_sb[:, 0:3], in_=w1[0])
            nc.scalar.dma_start(out=xT_sb[:, 0], in_=xt[0])
            nc.sync.dma_start(out=w1_sb[:, 3:6], in_=w1[1])
            nc.scalar.dma_start(out=xT_sb[:, 1], in_=xt[1])
            nc.sync.dma_start(out=b1_sb[:], in_=b1[:])
            nc.scalar.dma_start(out=wout_sb[:], in_=wout[:])

            # h^T[e, m] = relu(sum_d W1[d, e] * xT[d, m] + b1[e]), fp16.
            # c (m-half) outer: the first six PSUM groups touch only the
            # first xt half, which lands ~3.5us before the second.
            h_sb = pool.tile([128, ET, MS], dt.float16)
            for c in range(2):
                for et in range(ET):
                    ps = psh.tile([128, 512], dt.float32)
                    for dti in range(ET):
                        nc.tensor.matmul(
                            ps[:],
                            w1_sb[:, et, dti, :],
                            xT_sb[:, c, dti, :],
                            start=(dti == 0),
                            stop=(dti == ET - 1),
                        )
                    nc.scalar.activation(
                        h_sb[:, et, 512 * c : 512 * (c + 1)],
                        ps[:],
                        mybir.ActivationFunctionType.Relu,
                        bias=b1_sb[:, et : et + 1],
                    )

            # s[1, m] = sum_e Wout[e] * h^T[e, m]; out exp(s) fp32
            exps_sb = pool.tile([1, MS], dt.float32)
            for c in range(2):
                ps2 = pss.tile([1, 512], dt.float32)
                for et in range(ET):
                    nc.tensor.matmul(
                        ps2[:],
                        wout_sb[:, et : et + 1],
                        h_sb[:, et, 512 * c : 512 * (c + 1)],
                        start=(et == 0),
                        stop=(et == ET - 1),
                    )
                nc.scalar.activation(
                    exps_sb[:, 512 * c : 512 * (c + 1)],
                    ps2[:],
                    mybir.ActivationFunctionType.Exp,
                )
            nc.sync.dma_start(out=exps[:], in_=exps_sb[:])
    nc.compile()
    return nc


def _build_phase2(gcap: int) -> bass.Bass:
    """Per-core: gather `2*gcap` compacted aug rows, masked weighted sums.

    The (c, l) slot stream is host-compacted to valid slots only (mean
    cluster length is ~8.5 of 16, so this nearly halves the gather
    descriptor count — the SWDGE generation rate is phase 2's floor).
    Each 128-concept output group gets a fixed budget of `gcap` slots,
    padded with sentinel (all-zeros row) indices.  The slot->concept map
    is a host-built 0/1 matrix per k-tile (msk); lhsT = msk * exp-score.
    """
    cap = NG * gcap          # total slots per core
    tpg = gcap // 128        # k-tiles per concept group
    tt = NG * tpg            # total k-tiles
    # uneven gather chunks, big -> small: the last chunk's descriptor
    # generation and transfer serialize at the end of the kernel, so keep
    # them short.
    if tt >= 8:
        c0 = -(-tt * 3 // 9)
        c2 = max(1, tt * 2 // 9)
        chunk_tiles = [c0, c0, c2, tt - 2 * c0 - c2]
    else:
        chunk_tiles = [1] * tt
    nchunks = len(chunk_tiles)
    assert all(ct > 0 for ct in chunk_tiles) and sum(chunk_tiles) == tt
    tile2chunk = []
    for j, ct in enumerate(chunk_tiles):
        tile2chunk += [j] * ct
    chunk_tile0 = [sum(chunk_tiles[:j]) for j in range(nchunks)]

    nc = _new_bass()
    aug = nc.declare_dram_parameter("aug", [M + 1, AUGW], dt.float16, isOutput=False)
    idxp = nc.declare_dram_parameter("idx", [128, cap // 16], dt.int16, isOutput=False)
    mskp = nc.declare_dram_parameter("msk", [128, NG * tpg * 128], dt.float16,
                                     isOutput=False)
    out = nc.declare_dram_parameter("out", [CS, D], dt.float32, isOutput=True)

    with tile.TileContext(nc) as tc:
        with (
            tc.tile_pool(name="sb", bufs=1) as pool,
            tc.tile_pool(name="g", bufs=1) as gpool,
            tc.tile_pool(name="z", bufs=4) as zpool,
            tc.tile_pool(name="ps1", bufs=2, space=bass.MemorySpace.PSUM) as ps1pool,
            tc.tile_pool(name="ps2", bufs=2, space=bass.MemorySpace.PSUM) as ps2pool,
            tc.tile_pool(name="psj", bufs=1, space=bass.MemorySpace.PSUM) as psj,
        ):
            idx_sb = pool.tile([128, cap // 16], dt.int16)
            # chunk-0 indices land first so gather 0 can start the moment the
            # Q7 ucode IRAM load finishes
            s1 = chunk_tiles[0] * 8
            nc.sync.dma_start(out=idx_sb[:, 0:s1], in_=idxp[:, 0:s1])
            nc.sync.dma_start(out=idx_sb[:, s1:], in_=idxp[:, s1:])
            msk_sb = pool.tile([128, NG * tpg, 128], dt.float16)
            nc.sync.dma_start(out=msk_sb[:], in_=mskp[:])

            # Gathers next in gpsimd program order: nothing else may clog the
            # Pool engine before them.
            gts = []
            for j in range(nchunks):
                chi = chunk_tiles[j] * 128
                gt = gpool.tile([128, chunk_tiles[j], AUGW], dt.float16,
                                name=f"gt{j}", tag=f"gt{j}")
                nc.gpsimd.dma_gather(
                    gt[:],
                    aug[:],
                    idx_sb[:, chunk_tile0[j] * 8 : chunk_tile0[j] * 8 + chi // 16],
                    chi,
                    chi,
                    AUGW,
                    single_packet=True,
                )
                gts.append(gt)

            wt, jps = _warmup(nc, pool, psj, 12)

            pcol_all = pool.tile([128, NG * tpg], dt.float32)
            for g in range(NG):
                ps1 = ps1pool.tile([128, 512], dt.float32)
                ps2 = ps2pool.tile([128, 258], dt.float32)
                for t in range(tpg):
                    T = tpg * g + t
                    j = tile2chunk[T]
                    gt, ti = gts[j], T - chunk_tile0[j]
                    if 0 < j < nchunks - 1 and ti == 0:
                        # HAM anchor: junk matmuls when a new chunk lands so
                        # the real matmuls run at 2.4 GHz (not on the last
                        # chunk — there they'd delay the final real matmuls).
                        for i in range(6):
                            nc.tensor.matmul(jps[i % 2][:], wt[:, 0:128],
                                             gt[:, 0, 0:512], start=True,
                                             stop=True, skip_group_check=True)
                    # lhsT[r, m] = exp-score(slot 128T+r) * msk[r, m]
                    nc.vector.tensor_copy(
                        pcol_all[:, T : T + 1], gt[:, ti, D : D + 1]
                    )
                    zt = zpool.tile([128, 128], dt.float16)
                    nc.vector.tensor_scalar(
                        zt[:],
                        msk_sb[:, T, :],
                        pcol_all[:, T : T + 1],
                        None,
                        mybir.AluOpType.mult,
                    )
                    nc.tensor.matmul(
                        ps1[:],
                        zt[:],
                        gt[:, ti, 0:512],
                        start=(t == 0),
                        stop=(t == tpg - 1),
                        skip_group_check=True,
                    )
                    nc.tensor.matmul(
                        ps2[:],
                        zt[:],
                        gt[:, ti, 512 : D + 2],
                        start=(t == 0),
                        stop=(t == tpg - 1),
                        skip_group_check=True,
                    )
                # ps1[:, 0:512] = unnorm out cols 0:512
                # ps2[:, 0:256] = unnorm out cols 512:768; ps2[:, 257] = denom
                rinv = pool.tile([128, 1], dt.float32, tag=f"rinv{g}")
                nc.vector.reciprocal(rinv[:], ps2[:, 257:258])
                out_sb = pool.tile([128, D], dt.float32, tag=f"os{g}")
                nc.vector.tensor_scalar(
                    out_sb[:, 0:512], ps1[:], rinv[:], None, mybir.AluOpType.mult
                )
                nc.vector.tensor_scalar(
                    out_sb[:, 512:D], ps2[:, 0:256], rinv[:], None, mybir.AluOpType.mult
                )
                nc.sync.dma_start(out=out[128 * g : 128 * (g + 1), :], in_=out_sb[:])
    nc.compile()
    return nc


def _get_programs(gcap):
    if "p1" not in _PROGRAMS:
        _PROGRAMS["p1"] = _build_phase1()
    key = f"p2-{gcap}"
    if key not in _PROGRAMS:
        _PROGRAMS[key] = _build_phase2(gcap)
    return _PROGRAMS["p1"], _PROGRAMS[key]


def _balance(cl):
    """Assign concepts to the 16 (core, group) bins, 128 each, minimizing the
    max bin load (sum of lengths).  Greedy longest-first.  Returns perm with
    perm[bin*128 + i] = original concept id."""
    nbins = N_CORES * NG
    order = np.argsort(-cl, kind="stable")
    loads = np.zeros(nbins)
    fill = np.zeros(nbins, np.int64)
    perm = np.empty((nbins, 128), np.int64)
    for c in order:
        open_bins = np.nonzero(fill < 128)[0]
        b = open_bins[np.argmin(loads[open_bins])]
        perm[b, fill[b]] = c
        fill[b] += 1
        loads[b] += cl[c]
    return perm.reshape(-1)


def _compact_slots(ci, cl):
    """Per-core compacted slot streams + per-tile concept masks.

    Returns (gcap, idx_streams[8][cap], masks[8][128, NG*tpg*128]).
    gcap = slot budget per 128-concept group (multiple of 128, shared by all
    cores so one compiled program serves all of them).
    """
    counts = cl.reshape(N_CORES * NG, 128).sum(axis=1)  # valid slots per group
    gcap = int(-(-counts.max() // 128) * 128)
    tpg = gcap // 128
    cap = NG * gcap
    idx_streams = np.full((N_CORES, cap), SENT, np.int16)
    masks = np.zeros((N_CORES, 128, NG * tpg, 128), np.float16)
    for k in range(N_CORES):
        for g in range(NG):
            base_c = CS * k + 128 * g
            pos = 0
            for m in range(128):
                c = base_c + m
                ln = int(cl[c])
                s0 = g * gcap + pos
                idx_streams[k, s0 : s0 + ln] = ci[c, :ln]
                for s in range(s0, s0 + ln):
                    masks[k, s % 128, s // 128, m] = 1.0
                pos += ln
    return gcap, idx_streams, masks.reshape(N_CORES, 128, NG * tpg * 128)


def _phase1_in_maps(flat, W1, b1, Wout):
    f16 = np.float16
    # w1[h, p, (j*ET + dti)*128 + e] = W1[128*dti + p, 128*(3h + j) + e]
    w1v = W1.reshape(ET, 128, ET, 128).transpose(2, 1, 0, 3)  # [et, p, dti, e]
    w1t = np.ascontiguousarray(
        w1v.reshape(2, 3, 128, ET, 128).transpose(0, 2, 1, 3, 4).reshape(
            2, 128, 3 * ET * 128
        )
    ).astype(f16)
    b1_l = np.ascontiguousarray(b1.reshape(ET, 128).T).astype(np.float32)
    wout_l = np.ascontiguousarray(Wout.reshape(ET, 128).T).astype(f16)
    maps = []
    for k in range(N_CORES):
        shard = flat[MS * k : MS * (k + 1)]                    # [1024, 768]
        # xt[c, p, t*512 + m] = shard[512c + m, 128t + p]
        v = shard.T.reshape(ET, 128, 2, 512)                   # [t, p, c, m]
        xtk = np.ascontiguousarray(
            v.transpose(2, 1, 0, 3).reshape(2, 128, ET * 512)
        ).astype(f16)
        maps.append({"xt": xtk, "w1": w1t, "b1": b1_l, "wout": wout_l})
    return maps


def _phase2_in_maps(flat, exps, ci, cl):
    aug = np.zeros((M + 1, AUGW), np.float16)
    aug[:M, :D] = flat.astype(np.float16)
    # Positive rescale cancels in numerator and denominator; keeps the fp16
    # exp column in range no matter the score distribution.
    aug[:M, D] = (exps / exps.max()).astype(np.float16)
    aug[:M, D + 1] = 1.0

    gcap, idx_streams, masks = _compact_slots(ci, cl)
    cap = NG * gcap

    maps = []
    for k in range(N_CORES):
        idxk = idx_streams[k]
        wrapped = np.ascontiguousarray(np.tile(idxk.reshape(cap // 16, 16).T, (8, 1)))
        maps.append({"aug": aug, "idx": wrapped,
                     "msk": np.ascontiguousarray(masks[k])})
    return gcap, maps


def kernel(
    mention_vectors,
    concept_indices,
    concept_lengths,
    W1,
    b1,
    Wout,
    bout,
    _trace=False,
):
    mv = np.ascontiguousarray(np.asarray(mention_vectors, dtype=np.float32))
    ci = np.asarray(concept_indices).astype(np.int64)
    cl = np.asarray(concept_lengths).astype(np.int64)
    W1 = np.asarray(W1, dtype=np.float32)
    b1 = np.asarray(b1, dtype=np.float32)
    Wout = np.asarray(Wout, dtype=np.float32)
    # bout cancels in the softmax; unused.
    flat = mv.reshape(M, D)

    # Load-balance concepts across (core, group) bins; host un-permutes the
    # output at the end.
    perm = _balance(cl)
    ci_p, cl_p = ci[perm], cl[perm]
    counts = cl_p.reshape(N_CORES * NG, 128).sum(axis=1)
    gcap0 = int(-(-counts.max() // 128) * 128)
    nc1, nc2 = _get_programs(gcap0)
    core_ids = list(range(N_CORES))

    r1 = run_bass_kernel_spmd(nc1, _phase1_in_maps(flat, W1, b1, Wout), core_ids,
                              trace=_trace)
    exps = np.concatenate(
        [r1.results[k]["exps"].reshape(MS) for k in range(N_CORES)]
    )

    gcap, maps2 = _phase2_in_maps(flat, exps, ci_p, cl_p)
    assert gcap == gcap0
    r2 = run_bass_kernel_spmd(nc2, maps2, core_ids, trace=_trace)
    out_p = np.concatenate([r2.results[k]["out"] for k in range(N_CORES)], axis=0)
    out = np.empty_like(out_p)
    out[perm] = out_p
    result = out.reshape(B, C, D).astype(np.float32)
    if _trace:
        return result, (r1, r2)
    return result


# revision 57
# speedup vs baseline: 1.0435x; 1.0435x over previous
"""Trainium2 Bass kernel for AggregateSelfAttention (ragged clusters).

Math (reference):
    flat = mention_vectors.reshape(8192, 768)
    v[c,l,:] = flat[idx[c,l]]
    s[c,l]   = relu(v @ W1 + b1) @ Wout + bout
    p        = softmax(mask(s))
    out[c]   = sum_l p[c,l] * v[c,l]

Key restructurings (validated vs reference at ~3e-4 rel l2 in fp16):
  * The score s[c,l] depends only on the mention row -> compute the FFN once
    per table row (8192 rows, sharded 1024/core) instead of per (c,l)
    occurrence (32768 rows): 4x less matmul work.  bout drops out entirely
    (softmax shift invariance).
  * Unnormalized softmax: p = exp(s)*valid / sum_l exp(s)*valid.  exp is safe
    without max subtraction (|s| < ~5 for unit-normal data; fp32 exp).
  * Host builds an augmented fp16 table  aug[m] = [flat[m] | exp(s_m) | 1 | pad]
    (row = 896 fp16 = 1792B, 256B-aligned for dma_gather).  Padded (c,l) slots
    point at a sentinel all-zeros row, so no masking is needed on device: the
    zero "exp" and zero "1" columns contribute nothing to numerator or
    denominator.
  * Phase 2 gathers the compacted valid (c, l) slots (~2300 rows/core after
    host load-balancing of concepts across cores, vs 4096 naive) with SWDGE
    dma_gather (1792B/descriptor) and does the ragged weighted sums as
    masked PE matmuls: lhsT[128,128] = host-built 0/1 slot->concept mask x
    per-row exp(s); rhs = gathered rows; the "1" column yields the softmax
    denominator in the same matmul.  One DVE reciprocal+scale normalizes.

Sharding: concepts (2048 -> 256/core) for phase 2; table rows (8192 ->
1024/core) for phase 1.  No collectives: phase 1 outputs exp-scores which the
host concatenates into the phase-2 table (pure data movement between the two
NEFF dispatches).

Perf notes (from NTFF traces):
  * Host pre-tiles all phase-1 operands into [128, free] partition-major
    contiguous blocks so each DMA descriptor is multi-KB (the naive
    rearranged DMA ran at 86 GB/s on 1.5KB descriptors).
  * Junk warmup matmuls at kernel start (and gated on mid gather chunks in
    phase 2) keep the PE HAM clock at 2.4 GHz; without them every matmul runs
    at the cold 1.2 GHz rate.
  * The gather's SWDGE descriptor generation (~8.6 ns/row, serial on the Q7,
    plus a fixed ~7us engine preamble and ~8-10us one-time ucode IRAM load)
    is phase 2's floor; everything else hides under it.  Gather chunks are
    sized big->small so the final chunk's generation + transfer tail is
    short.
"""

import os
import sys

import numpy as np

for _p in ("/opt/trn_rl_repo", "/root/.axon_site/_ro/trn_rl_repo"):
    if os.path.isdir(_p) and _p not in sys.path:
        sys.path.insert(0, _p)

from concourse import bacc, bass, mybir, tile  # noqa: E402
from concourse.bass_utils import run_bass_kernel_spmd  # noqa: E402

dt = mybir.dt

N_CORES = 8
B, M, D, C, L = 1, 8192, 768, 2048, 16
MS = M // N_CORES            # 1024 table rows per core (phase 1)
CS = C // N_CORES            # 256 concepts per core (phase 2)
NI = CS * L                  # 4096 gathered rows per core
AUGW = 896                   # fp16 aug row: 768 X | exp | 1 | pad -> 1792B
SENT = M                     # sentinel row index (all zeros)
ET = D // 128                # 6 partition tiles of the 768 dim
NG = CS // 128               # 2 concept groups of 128

_PROGRAMS = {}


def _new_bass(num_swdge_queues: int = 1) -> bacc.Bacc:
    return bacc.Bacc(
        "TRN2",
        target_bir_lowering=False,
        debug=False,
        num_devices=N_CORES,
        num_swdge_queues=num_swdge_queues,
    )


def _warmup(nc, pool, psum_pool, n_mm):
    """Junk matmuls to push the PE HAM clock to 2.4 GHz while DMAs run.

    Two alternating PSUM banks so the junk matmuls pipeline instead of
    serializing on a same-bank WAW drain.
    """
    wt = pool.tile([128, 512], dt.float16, tag="warm")
    nc.vector.memset(wt[:], 0.0)
    jps = [
        psum_pool.tile([128, 512], dt.float32, name="jp0", tag="jp0"),
        psum_pool.tile([128, 512], dt.float32, name="jp1", tag="jp1"),
    ]
    for i in range(n_mm):
        nc.tensor.matmul(jps[i % 2][:], wt[:, 0:128], wt[:], start=True,
                         stop=True, skip_group_check=True)
    return wt, jps


def _build_phase1() -> bass.Bass:
    """Per-core: exp(relu(X_shard @ W1 + b1) @ Wout) for 1024 table rows.

    Host-pretiled inputs (partition-major, contiguous per partition):
      xt[c, p, t*512 + m] = X_shard[512c + m, 128t + p]      (c: m-half)
      w1[h, p, ((j*ET + dti)*128) + e] = W1[128*dti + p, 128*(3h+j) + e]
      b1[p, et] = b1[128*et + p];  wout[p, et] = Wout[128*et + p]
    """
    nc = _new_bass()
    xt = nc.declare_dram_parameter("xt", [2, 128, ET * 512], dt.float16, isOutput=False)
    w1 = nc.declare_dram_parameter("w1", [2, 128, 3 * ET * 128], dt.float16, isOutput=False)
    b1 = nc.declare_dram_parameter("b1", [128, ET], dt.float32, isOutput=False)
    wout = nc.declare_dram_parameter("wout", [128, ET], dt.float16, isOutput=False)
    exps = nc.declare_dram_parameter("exps", [1, MS], dt.float32, isOutput=True)

    with tile.TileContext(nc) as tc:
        with (
            tc.tile_pool(name="sb", bufs=1) as pool,
            tc.tile_pool(name="psh", bufs=3, space=bass.MemorySpace.PSUM) as psh,
            tc.tile_pool(name="pss", bufs=1, space=bass.MemorySpace.PSUM) as pss,
            tc.tile_pool(name="psj", bufs=1, space=bass.MemorySpace.PSUM) as psj,
        ):
            _warmup(nc, pool, psj, 12)

            w1_sb = pool.tile([128, ET, ET, 128], dt.float16)  # [p, et, dti, e]
            xT_sb = pool.tile([128, 2, ET, 512], dt.float16)   # [p, c, t, m]
            b1_sb = pool.tile([128, ET], dt.float32)
            wout_sb = pool.tile([128, ET], dt.float16)
            # order matters: first compute needs w1 half 0 + xt half 0.
            # Two HWDGE rings (sync + scalar) feed the SDMA engines in
            # parallel to cut the latency to the first matmul's operands.
            nc.sync.dma_start(out=w1_sb[:, 0:3], in_=w1[0])
            nc.scalar.dma_start(out=xT_sb[:, 0], in_=xt[0])
            nc.sync.dma_start(out=w1_sb[:, 3:6], in_=w1[1])
            nc.scalar.dma_start(out=xT_sb[:, 1], in_=xt[1])
            nc.sync.dma_start(out=b1_sb[:], in_=b1[:])
            nc.scalar.dma_start(out=wout_sb[:], in_=wout[:])

            # h^T[e, m] = relu(sum_d W1[d, e] * xT[d, m] + b1[e]), fp16.
            # c (m-half) outer: the first six PSUM groups touch only the
            # first xt half, which lands ~3.5us before the second.
            h_sb = pool.tile([128, ET, MS], dt.float16)
            for c in range(2):
                for et in range(ET):
                    ps = psh.tile([128, 512], dt.float32)
                    for dti in range(ET):
                        nc.tensor.matmul(
                            ps[:],
                            w1_sb[:, et, dti, :],
                            xT_sb[:, c, dti, :],
                            start=(dti == 0),
                            stop=(dti == ET - 1),
                        )
                    nc.scalar.activation(
                        h_sb[:, et, 512 * c : 512 * (c + 1)],
                        ps[:],
                        mybir.ActivationFunctionType.Relu,
                        bias=b1_sb[:, et : et + 1],
                    )

            # s[1, m] = sum_e Wout[e] * h^T[e, m]; out exp(s) fp32
            exps_sb = pool.tile([1, MS], dt.float32)
            for c in range(2):
                ps2 = pss.tile([1, 512], dt.float32)
                for et in range(ET):
                    nc.tensor.matmul(
                        ps2[:],
                        wout_sb[:, et : et + 1],
                        h_sb[:, et, 512 * c : 512 * (c + 1)],
                        start=(et == 0),
                        stop=(et == ET - 1),
                    )
                nc.scalar.activation(
                    exps_sb[:, 512 * c : 512 * (c + 1)],
                    ps2[:],
                    mybir.ActivationFunctionType.Exp,
                )
            nc.sync.dma_start(out=exps[:], in_=exps_sb[:])
    nc.compile()
    return nc


def _build_phase2(gcap: int) -> bass.Bass:
    """Per-core: gather `2*gcap` compacted aug rows, masked weighted sums.

    The (c, l) slot stream is host-compacted to valid slots only (mean
    cluster length is ~8.5 of 16, so this nearly halves the gather
    descriptor count — the SWDGE generation rate is phase 2's floor).
    Each 128-concept output group gets a fixed budget of `gcap` slots,
    padded with sentinel (all-zeros row) indices.  The slot->concept map
    is a host-built 0/1 matrix per k-tile (msk); lhsT = msk * exp-score.
    """
    cap = NG * gcap          # total slots per core
    tpg = gcap // 128        # k-tiles per concept group
    tt = NG * tpg            # total k-tiles
    # Gather chunks big -> small, alternating between the two SWDGE queues
    # (chunks 0,2 -> queue 0; 1,3 -> queue 1).  Sizes balance the per-queue
    # byte totals so neither ring finishes late, with a small final chunk
    # since its generation + transfer + matmuls form the kernel tail.
    if tt >= 8:
        half = tt // 2
        d = max(1, tt // 9)
        b = half - d
        c = max(1, tt // 6)
        a = tt - half - c
        chunk_tiles = [a, b, c, d]
    else:
        chunk_tiles = [1] * tt
    nchunks = len(chunk_tiles)
    assert all(ct > 0 for ct in chunk_tiles) and sum(chunk_tiles) == tt
    tile2chunk = []
    for j, ct in enumerate(chunk_tiles):
        tile2chunk += [j] * ct
    chunk_tile0 = [sum(chunk_tiles[:j]) for j in range(nchunks)]

    nc = _new_bass(num_swdge_queues=2)
    aug = nc.declare_dram_parameter("aug", [M + 1, AUGW], dt.float16, isOutput=False)
    idxp = nc.declare_dram_parameter("idx", [128, cap // 16], dt.int16, isOutput=False)
    mskp = nc.declare_dram_parameter("msk", [128, NG * tpg * 128], dt.float16,
                                     isOutput=False)
    out = nc.declare_dram_parameter("out", [CS, D], dt.float32, isOutput=True)

    with tile.TileContext(nc) as tc:
        with (
            tc.tile_pool(name="sb", bufs=1) as pool,
            tc.tile_pool(name="g", bufs=1) as gpool,
            tc.tile_pool(name="z", bufs=4) as zpool,
            tc.tile_pool(name="ps1", bufs=2, space=bass.MemorySpace.PSUM) as ps1pool,
            tc.tile_pool(name="ps2", bufs=2, space=bass.MemorySpace.PSUM) as ps2pool,
            tc.tile_pool(name="psj", bufs=1, space=bass.MemorySpace.PSUM) as psj,
        ):
            # One idx tile per chunk: each gather's semaphore wait covers only
            # its own DMA, so gather 0 fires the moment the Q7 ucode IRAM load
            # finishes.  msk rides the other HWDGE ring (scalar) to keep the
            # sync ring clear for the indices.
            idx_js = []
            for j in range(nchunks):
                ij = pool.tile([128, chunk_tiles[j] * 8], dt.int16,
                               name=f"idx{j}", tag=f"idx{j}")
                nc.sync.dma_start(
                    out=ij[:],
                    in_=idxp[:, chunk_tile0[j] * 8 : (chunk_tile0[j] + chunk_tiles[j]) * 8],
                )
                idx_js.append(ij)
            msk_sb = pool.tile([128, NG * tpg, 128], dt.float16)
            nc.scalar.dma_start(out=msk_sb[:], in_=mskp[:])

            # Gathers next in gpsimd program order: nothing else may clog the
            # Pool engine before them.  Alternating SWDGE queues let chunk
            # j+1's SDMA transfer overlap chunk j's instead of serializing
            # on one qPoolDynamic ring.
            gts = []
            for j in range(nchunks):
                chi = chunk_tiles[j] * 128
                gt = gpool.tile([128, chunk_tiles[j], AUGW], dt.float16,
                                name=f"gt{j}", tag=f"gt{j}")
                nc.gpsimd.dma_gather(
                    gt[:],
                    aug[:],
                    idx_js[j][:],
                    chi,
                    chi,
                    AUGW,
                    single_packet=True,
                    queue_num=j % 2,
                )
                gts.append(gt)

            wt, jps = _warmup(nc, pool, psj, 12)

            pcol_all = pool.tile([128, NG * tpg], dt.float32)
            for g in range(NG):
                ps1 = ps1pool.tile([128, 512], dt.float32)
                ps2 = ps2pool.tile([128, 258], dt.float32)
                for t in range(tpg):
                    T = tpg * g + t
                    j = tile2chunk[T]
                    gt, ti = gts[j], T - chunk_tile0[j]
                    if 0 < j < nchunks - 1 and ti == 0:
                        # HAM anchor: junk matmuls when a new chunk lands so
                        # the real matmuls run at 2.4 GHz (not on the last
                        # chunk — there they'd delay the final real matmuls).
                        for i in range(6):
                            nc.tensor.matmul(jps[i % 2][:], wt[:, 0:128],
                                             gt[:, 0, 0:512], start=True,
                                             stop=True, skip_group_check=True)
                    # lhsT[r, m] = exp-score(slot 128T+r) * msk[r, m]
                    nc.vector.tensor_copy(
                        pcol_all[:, T : T + 1], gt[:, ti, D : D + 1]
                    )
                    zt = zpool.tile([128, 128], dt.float16)
                    nc.vector.tensor_scalar(
                        zt[:],
                        msk_sb[:, T, :],
                        pcol_all[:, T : T + 1],
                        None,
                        mybir.AluOpType.mult,
                    )
                    nc.tensor.matmul(
                        ps1[:],
                        zt[:],
                        gt[:, ti, 0:512],
                        start=(t == 0),
                        stop=(t == tpg - 1),
                        skip_group_check=True,
                    )
                    nc.tensor.matmul(
                        ps2[:],
                        zt[:],
                        gt[:, ti, 512 : D + 2],
                        start=(t == 0),
                        stop=(t == tpg - 1),
                        skip_group_check=True,
                    )
                # ps1[:, 0:512] = unnorm out cols 0:512
                # ps2[:, 0:256] = unnorm out cols 512:768; ps2[:, 257] = denom
                rinv = pool.tile([128, 1], dt.float32, tag=f"rinv{g}")
                nc.vector.reciprocal(rinv[:], ps2[:, 257:258])
                out_sb = pool.tile([128, D], dt.float32, tag=f"os{g}")
                nc.vector.tensor_scalar(
                    out_sb[:, 0:512], ps1[:], rinv[:], None, mybir.AluOpType.mult
                )
                nc.vector.tensor_scalar(
                    out_sb[:, 512:D], ps2[:, 0:256], rinv[:], None, mybir.AluOpType.mult
                )
                nc.sync.dma_start(out=out[128 * g : 128 * (g + 1), :], in_=out_sb[:])
    nc.compile()
    return nc


def _get_programs(gcap):
    if "p1" not in _PROGRAMS:
        _PROGRAMS["p1"] = _build_phase1()
    key = f"p2-{gcap}"
    if key not in _PROGRAMS:
        _PROGRAMS[key] = _build_phase2(gcap)
    return _PROGRAMS["p1"], _PROGRAMS[key]


def _balance(cl):
    """Assign concepts to the 16 (core, group) bins, 128 each, minimizing the
    max bin load (sum of lengths).  Greedy longest-first.  Returns perm with
    perm[bin*128 + i] = original concept id."""
    nbins = N_CORES * NG
    order = np.argsort(-cl, kind="stable")
    loads = np.zeros(nbins)
    fill = np.zeros(nbins, np.int64)
    perm = np.empty((nbins, 128), np.int64)
    for c in order:
        open_bins = np.nonzero(fill < 128)[0]
        b = open_bins[np.argmin(loads[open_bins])]
        perm[b, fill[b]] = c
        fill[b] += 1
        loads[b] += cl[c]
    return perm.reshape(-1)


def _compact_slots(ci, cl):
    """Per-core compacted slot streams + per-tile concept masks.

    Returns (gcap, idx_streams[8][cap], masks[8][128, NG*tpg*128]).
    gcap = slot budget per 128-concept group (multiple of 128, shared by all
    cores so one compiled program serves all of them).
    """
    counts = cl.reshape(N_CORES * NG, 128).sum(axis=1)  # valid slots per group
    gcap = int(-(-counts.max() // 128) * 128)
    tpg = gcap // 128
    cap = NG * gcap
    idx_streams = np.full((N_CORES, cap), SENT, np.int16)
    masks = np.zeros((N_CORES, 128, NG * tpg, 128), np.float16)
    for k in range(N_CORES):
        for g in range(NG):
            base_c = CS * k + 128 * g
            pos = 0
            for m in range(128):
                c = base_c + m
                ln = int(cl[c])
                s0 = g * gcap + pos
                idx_streams[k, s0 : s0 + ln] = ci[c, :ln]
                for s in range(s0, s0 + ln):
                    masks[k, s % 128, s // 128, m] = 1.0
                pos += ln
    return gcap, idx_streams, masks.reshape(N_CORES, 128, NG * tpg * 128)


def _phase1_in_maps(flat, W1, b1, Wout):
    f16 = np.float16
    # w1[h, p, (j*ET + dti)*128 + e] = W1[128*dti + p, 128*(3h + j) + e]
    w1v = W1.reshape(ET, 128, ET, 128).transpose(2, 1, 0, 3)  # [et, p, dti, e]
    w1t = np.ascontiguousarray(
        w1v.reshape(2, 3, 128, ET, 128).transpose(0, 2, 1, 3, 4).reshape(
            2, 128, 3 * ET * 128
        )
    ).astype(f16)
    b1_l = np.ascontiguousarray(b1.reshape(ET, 128).T).astype(np.float32)
    wout_l = np.ascontiguousarray(Wout.reshape(ET, 128).T).astype(f16)
    maps = []
    for k in range(N_CORES):
        shard = flat[MS * k : MS * (k + 1)]                    # [1024, 768]
        # xt[c, p, t*512 + m] = shard[512c + m, 128t + p]
        v = shard.T.reshape(ET, 128, 2, 512)                   # [t, p, c, m]
        xtk = np.ascontiguousarray(
            v.transpose(2, 1, 0, 3).reshape(2, 128, ET * 512)
        ).astype(f16)
        maps.append({"xt": xtk, "w1": w1t, "b1": b1_l, "wout": wout_l})
    return maps


def _phase2_in_maps(flat, exps, ci, cl):
    aug = np.zeros((M + 1, AUGW), np.float16)
    aug[:M, :D] = flat.astype(np.float16)
    # Positive rescale cancels in numerator and denominator; keeps the fp16
    # exp column in range no matter the score distribution.
    aug[:M, D] = (exps / exps.max()).astype(np.float16)
    aug[:M, D + 1] = 1.0

    gcap, idx_streams, masks = _compact_slots(ci, cl)
    cap = NG * gcap

    maps = []
    for k in range(N_CORES):
        idxk = idx_streams[k]
        wrapped = np.ascontiguousarray(np.tile(idxk.reshape(cap // 16, 16).T, (8, 1)))
        maps.append({"aug": aug, "idx": wrapped,
                     "msk": np.ascontiguousarray(masks[k])})
    return gcap, maps


def kernel(
    mention_vectors,
    concept_indices,
    concept_lengths,
    W1,
    b1,
    Wout,
    bout,
    _trace=False,
):
    mv = np.ascontiguousarray(np.asarray(mention_vectors, dtype=np.float32))
    ci = np.asarray(concept_indices).astype(np.int64)
    cl = np.asarray(concept_lengths).astype(np.int64)
    W1 = np.asarray(W1, dtype=np.float32)
    b1 = np.asarray(b1, dtype=np.float32)
    Wout = np.asarray(Wout, dtype=np.float32)
    # bout cancels in the softmax; unused.
    flat = mv.reshape(M, D)

    # Load-balance concepts across (core, group) bins; host un-permutes the
    # output at the end.
    perm = _balance(cl)
    ci_p, cl_p = ci[perm], cl[perm]
    counts = cl_p.reshape(N_CORES * NG, 128).sum(axis=1)
    gcap0 = int(-(-counts.max() // 128) * 128)
    nc1, nc2 = _get_programs(gcap0)
    core_ids = list(range(N_CORES))

    r1 = run_bass_kernel_spmd(nc1, _phase1_in_maps(flat, W1, b1, Wout), core_ids,
                              trace=_trace)
    exps = np.concatenate(
        [r1.results[k]["exps"].reshape(MS) for k in range(N_CORES)]
    )

    gcap, maps2 = _phase2_in_maps(flat, exps, ci_p, cl_p)
    assert gcap == gcap0
    r2 = run_bass_kernel_spmd(nc2, maps2, core_ids, trace=_trace)
    out_p = np.concatenate([r2.results[k]["out"] for k in range(N_CORES)], axis=0)
    out = np.empty_like(out_p)
    out[perm] = out_p
    result = out.reshape(B, C, D).astype(np.float32)
    if _trace:
        return result, (r1, r2)
    return result
